# revision 1
# baseline (speedup 1.0000x reference)
"""MeshConv (Chebyshev graph conv, K=6) Trainium2 kernel, 8 NeuronCores.

Strategy: vertex (dst-row) sharding across the 8 cores with 8-batch "tokens"
(one token = all 8 batches' 64 features of one vertex = 512 values, bf16 for
gathers).  Per Chebyshev step: AllGather the bf16 token array, dma_gather
per-edge source tokens into a fixed slot grid, multiply-accumulate per
128-row dst tile on the TensorEngine with host-built [slots x rows] value
patterns (edge weights live in the stationary operand), then a fused DVE
recurrence update in fp32.  The dense projection folds W into block-diagonal
per-batch-pair matrices applied to xbar-transposed bf16 activations.
"""
import os
import sys
import time

sys.path.insert(0, '/opt/trn_rl_repo')

import numpy as np
import ml_dtypes

import concourse.bass as bass
import concourse.bacc as bacc
import concourse.mybir as mybir
import concourse.tile as tile_mod
from concourse.tile import TileContext
from concourse.bass_utils import run_bass_kernel_spmd

# ---------------------------------------------------------------- constants
B, FIN, K, FOUT = 8, 64, 6, 64
NCORE = 8
TOK = B * FIN              # 512 values per vertex token

# walrus in this environment accepts only 1 sync-wait per CTRL instruction:
# spread the Tile tail-drain's waits across preceding nops.
def _patched_drain_and_barrier(self, tick_clock, wait_clock):
    nop0 = self.nc.sync.nop(nofuse=True)
    wait_clock.add_sem_waits(nop0.ins, tile_mod.ScopedClock({None: tick_clock.global_clock}))
    si = nop0.ins.sync_info
    waits = list(si.on_wait) if si and si.on_wait else []
    if len(waits) > 1:
        si.on_wait = waits[:1]
        rest = waits[1:]
        while rest:
            n = self.nc.sync.nop(nofuse=True)
            nsi = n.ins.sync_info
            if nsi is None:
                n.ins.sync_info = mybir.SyncInfo(on_wait=rest[:1], on_update=[])
            else:
                nsi.on_wait = rest[:1]
            rest = rest[1:]
    self.nc.sync.drain()
    self.nc.all_engine_barrier()
    assert self.sems is not None
    popped = self.nc._tile_sem_poison_stack.pop()
    assert popped is self._sem_poison
    self.nc.clear_and_free_semaphores(list(self.sems.allocated().values()))
    self.nc.all_engine_barrier()


tile_mod.TileContext._drain_and_barrier = _patched_drain_and_barrier


class Cfg:
    """Geometry of the slot grid.  Everything derives from (M, CPT_A, CPT_B)."""

    def __init__(self, M, ntile_core, cpt_a, cpt_b, ga_call, gb_call):
        self.M = M                           # real vertex count
        self.NTILE_CORE = ntile_core         # 128-row dst tiles per core
        self.SLICE = 128 * ntile_core        # rows per core
        self.MPAD = NCORE * self.SLICE       # padded vertex positions
        self.NTILE = NCORE * ntile_core
        self.CPT_A = cpt_a                   # A-chunks per tile
        self.CPT_B = cpt_b                   # B-chunks per tile
        self.CPT = cpt_a + cpt_b
        self.NCH_A = cpt_a * ntile_core      # A chunks per core
        self.NCH_B = cpt_b * ntile_core
        self.NCH = self.CPT * ntile_core
        self.NIDX_A = self.NCH_A * 128
        self.NIDX_B = self.NCH_B * 128
        self.GA_CALL = ga_call               # idxs per A gather call
        self.GB_CALL = gb_call
        # int16 index split: call A covers positions [0, 32768); call B uses
        # base ASPLIT-BSHIFT... B base chosen so B indices stay in [0, 32768).
        self.ASPLIT = min(32768, self.MPAD)  # positions < ASPLIT reachable by A
        self.BBASE = max(0, self.MPAD - 32768)  # B call base row
        assert self.MPAD - self.BBASE <= 32768

    def a_calls(self):
        """List of (start_chunk, n_idx) for the A gather calls."""
        out = []
        ch = 0
        while ch * 128 < self.NIDX_A:
            n = min(self.GA_CALL, self.NIDX_A - ch * 128)
            out.append((ch, n))
            ch += n // 128
        return out

    def b_calls(self):
        out = []
        ch = 0
        while ch * 128 < self.NIDX_B:
            n = min(self.GB_CALL, self.NIDX_B - ch * 128)
            out.append((ch, n))
            ch += n // 128
        return out


CFG_FULL = Cfg(M=40000, ntile_core=40, cpt_a=7, cpt_b=2, ga_call=4096, gb_call=2048)


# ---------------------------------------------------------------- host prep
def build_graph_data(cfg, edge_rows, edge_cols, edge_vals):
    """Slot the edge list into the fixed per-tile chunk grid.

    Returns per-core idxA/idxB (wrapped int16), pattern array, and the
    vertex<->position permutation.
    """
    M, MPAD = cfg.M, cfg.MPAD
    er = np.asarray(edge_rows).astype(np.int64)
    ec = np.asarray(edge_cols).astype(np.int64)
    ev = np.asarray(edge_vals).astype(np.float32)
    E = er.shape[0]

    outdeg = np.bincount(ec, minlength=M)
    indeg = np.bincount(er, minlength=M)

    # Zone split: lowest out-degree vertices go to the B zone (positions >=
    # ASPLIT) so B-only edges per tile stay small.
    nb_real = max(0, MPAD - cfg.ASPLIT - (MPAD - M))  # real vertices in B zone
    na_real = M - nb_real
    order_by_out = np.argsort(outdeg, kind="stable")
    bverts = order_by_out[:nb_real]
    averts = order_by_out[nb_real:]

    ntile_a = cfg.ASPLIT // 128
    ntile_b = (MPAD - cfg.ASPLIT) // 128
    v2pos = np.full(M, -1, np.int64)
    # in-degree balance: sort desc by indeg, round-robin over zone tiles
    a_sorted = averts[np.argsort(-indeg[averts], kind="stable")]
    i = np.arange(na_real)
    v2pos[a_sorted] = 128 * (i % ntile_a) + (i // ntile_a)
    if nb_real:
        b_sorted = bverts[np.argsort(-indeg[bverts], kind="stable")]
        i = np.arange(nb_real)
        assert (i // ntile_b).max() < 128
        v2pos[b_sorted] = cfg.ASPLIT + 128 * (i % ntile_b) + (i // ntile_b)
    assert (v2pos >= 0).all()

    rpos = v2pos[er]
    cpos = v2pos[ec]
    tile = rpos // 128
    rloc = rpos % 128

    # Per tile, split edges between A chunks (src pos < ASPLIT) and B chunks
    # (src pos >= BBASE), respecting capacities.
    capA = cfg.CPT_A * 128
    capB = cfg.CPT_B * 128
    idxA = np.zeros((NCORE, cfg.NIDX_A), np.int16)
    idxB = np.zeros((NCORE, cfg.NIDX_B), np.int16)
    pat = np.zeros((NCORE, cfg.NCH, 128, 128), np.float32)

    order = np.lexsort((cpos, tile))   # group by tile; B-eligible sorted last
    er_s, tile_s, rloc_s, cpos_s, ev_s = er[order], tile[order], rloc[order], cpos[order], ev[order]
    tstart = np.searchsorted(tile_s, np.arange(cfg.NTILE + 1))

    for t in range(cfg.NTILE):
        lo, hi = tstart[t], tstart[t + 1]
        n = hi - lo
        if n > capA + capB:
            raise RuntimeError(f"tile {t} overflow: {n} edges > {capA + capB}")
        cp = cpos_s[lo:hi]
        rl = rloc_s[lo:hi]
        vv = ev_s[lo:hi]
        bmask = cp >= cfg.ASPLIT            # must go to B
        amask = cp < cfg.BBASE              # must go to A
        nB_only = int(bmask.sum())
        if nB_only > capB:
            raise RuntimeError(f"tile {t}: B-only {nB_only} > capB {capB}")
        needB = max(nB_only, n - capA)
        # promote flexible (mid-range) edges to B if A would overflow
        bsel = bmask.copy()
        if needB > nB_only:
            flex = np.flatnonzero(~bmask & ~amask)
            bsel[flex[: needB - nB_only]] = True
        asel = ~bsel
        nA, nB = int(asel.sum()), int(bsel.sum())
        assert nA <= capA and nB <= capB, (t, nA, nB)

        core = t // cfg.NTILE_CORE
        tl = t % cfg.NTILE_CORE
        # A slots
        s = np.arange(nA)
        chA = tl * cfg.CPT_A + s // 128
        slA = s % 128
        idxA[core, chA * 128 + slA] = cp[asel].astype(np.int16)
        pat[core, (tl * cfg.CPT + (s // 128)), slA, rl[asel]] = vv[asel]
        # B slots
        s = np.arange(nB)
        chB = tl * cfg.CPT_B + s // 128
        slB = s % 128
        idxB[core, chB * 128 + slB] = (cp[bsel] - cfg.BBASE).astype(np.int16)
        pat[core, (tl * cfg.CPT + cfg.CPT_A + (s // 128)), slB, rl[bsel]] = vv[bsel]

    def wrap(idx):
        # dma_gather layout: idx i -> partition i%16, free i//16, replicated x8
        n = idx.shape[1]
        a = idx.reshape(NCORE, n // 16, 16).transpose(0, 2, 1)  # [NCORE, 16, n/16]
        return np.tile(a, (1, 8, 1)).copy()

    return {
        "idxA_w": wrap(idxA),
        "idxB_w": wrap(idxB),
        "pat": pat.astype(ml_dtypes.bfloat16),
        "v2pos": v2pos,
    }


def build_w_blocks(W):
    """W [FIN*K, FOUT] -> per-k block-diagonal [128, 128] (2 batches/block)."""
    Wk = np.asarray(W).astype(np.float32).reshape(FIN, K, FOUT)  # [fin, k, fo]
    blocks = np.zeros((K, 128, 128), np.float32)
    for k in range(K):
        blocks[k, 0:64, 0:64] = Wk[:, k, :]
        blocks[k, 64:128, 64:128] = Wk[:, k, :]
    return blocks.astype(ml_dtypes.bfloat16)


def build_x0(cfg, x, v2pos):
    """x [B, M, FIN] -> per-core fp32 token slices [SLICE, TOK] (b-major)."""
    M = cfg.M
    tok = np.zeros((cfg.MPAD, TOK), np.float32)
    xt = np.transpose(np.asarray(x).astype(np.float32), (1, 0, 2)).reshape(M, TOK)
    tok[v2pos] = xt
    return tok.reshape(NCORE, cfg.SLICE, TOK)


# ---------------------------------------------------------------- device IR
def build_nc(cfg, repeat=1):
    nc = bacc.Bacc(None, target_bir_lowering=False, debug=False,
                   dynamic_dma_scratch_size=16384)
    dt = mybir.dt
    S, T = cfg.SLICE, cfg.NTILE_CORE

    x0loc = nc.declare_dram_parameter("x0loc", [S, TOK], dt.float32, isOutput=False)
    idxA = nc.declare_dram_parameter("idxA", [128, cfg.NIDX_A // 16], dt.int16, isOutput=False)
    idxB = nc.declare_dram_parameter("idxB", [128, cfg.NIDX_B // 16], dt.int16, isOutput=False)
    patd = nc.declare_dram_parameter("pat", [cfg.NCH * 128, 128], dt.bfloat16, isOutput=False)
    wblk = nc.declare_dram_parameter("wblk", [K * 128, 128], dt.bfloat16, isOutput=False)
    outp = nc.declare_dram_parameter("outp", [512, S], dt.float32, isOutput=True)

    contrib = [nc.dram_tensor(f"contrib{k}", [S, TOK], dt.bfloat16) for k in range(K)]
    gathered = [nc.dram_tensor(f"gathered{k}", [cfg.MPAD, TOK], dt.bfloat16,
                               addr_space="Shared") for k in range(1, K)]
    xf = [x0loc] + [nc.dram_tensor(f"xf{k}", [S, TOK], dt.float32) for k in range(1, K)]

    a_calls = cfg.a_calls()
    b_calls = cfg.b_calls()
    # map chunk -> (call index, slot-in-call)
    def chunk_map(calls):
        m = {}
        for ci, (ch0, n) in enumerate(calls):
            for j in range(n // 128):
                m[ch0 + j] = (ci, j)
        return m

    amap, bmap = chunk_map(a_calls), chunk_map(b_calls)
    ga_free = max(n // 128 for _, n in a_calls)
    gb_free = max(n // 128 for _, n in b_calls)

    with TileContext(nc) as tc:
        with (
            tc.tile_pool(name="io", bufs=1) as io,
            tc.tile_pool(name="ga", bufs=2) as gap,
            tc.tile_pool(name="gb", bufs=2) as gbp,
            tc.tile_pool(name="patp", bufs=3) as patp,
            tc.tile_pool(name="ev", bufs=3) as evp,
            tc.tile_pool(name="prj", bufs=2) as prjp,
            tc.tile_pool(name="ps", bufs=3, space="PSUM") as psp,
            tc.tile_pool(name="psj", bufs=2, space="PSUM") as psjp,
        ):
            # resident: gather indices + W blocks
            idxA_t = io.tile([128, cfg.NIDX_A // 16], dt.int16)
            nc.sync.dma_start(out=idxA_t[:], in_=idxA[:])
            idxB_t = io.tile([128, cfg.NIDX_B // 16], dt.int16)
            nc.sync.dma_start(out=idxB_t[:], in_=idxB[:])
            w_t = io.tile([128, K, 128], dt.bfloat16)
            nc.sync.dma_start(out=w_t[:], in_=wblk[:].rearrange("(k p) r -> p k r", p=128))

            def projection(k):
                # outp[j*128 + (2b'|fo), r] += sum_fin Wk x_k
                for j in range(4):
                    xT = prjp.tile([128, S], dt.bfloat16, tag="xT")
                    nc.sync.dma_start(out=xT[:], in_=contrib[k][:, j * 128:(j + 1) * 128],
                                      transpose=True)
                    for rc in range(S // 512):
                        pj = psjp.tile([128, 512], dt.float32, tag="pj")
                        nc.tensor.matmul(pj[:], w_t[:, k, :], xT[:, rc * 512:(rc + 1) * 512],
                                         start=True, stop=True)
                        acc = prjp.tile([128, 512], dt.float32, tag="acc")
                        nc.sync.dma_start(out=acc[:], in_=outp[j * 128:(j + 1) * 128,
                                                              rc * 512:(rc + 1) * 512])
                        acc2 = prjp.tile([128, 512], dt.float32, tag="acc2")
                        nc.vector.tensor_add(acc2[:], acc[:], pj[:])
                        nc.sync.dma_start(out=outp[j * 128:(j + 1) * 128,
                                                   rc * 512:(rc + 1) * 512], in_=acc2[:])

            def stage0():
                for g in range(0, T, 2):
                    nt = min(2, T - g)
                    t0 = evp.tile([128, nt, TOK], dt.float32, tag="s0f")
                    nc.sync.dma_start(out=t0[:], in_=x0loc[:].rearrange(
                        "(a p) f -> p a f", p=128)[:, g:g + nt, :])
                    t0b = evp.tile([128, nt, TOK], dt.bfloat16, tag="s0b")
                    nc.vector.tensor_copy(t0b[:], t0[:])
                    nc.sync.dma_start(out=contrib[0][:].rearrange(
                        "(a p) f -> p a f", p=128)[:, g:g + nt, :], in_=t0b[:])

            def cheb_step(k, gk):
                gk = gathered[k - 1]
                nc.gpsimd.collective_compute(
                    "AllGather", mybir.AluOpType.bypass,
                    replica_groups=[list(range(NCORE))],
                    ins=[contrib[k - 1][:]], outs=[gk[:]],
                )
                GA, GB = [], []
                for (ch0, n) in a_calls:
                    g = gap.tile([128, ga_free, TOK], dt.bfloat16, tag="ga")
                    nc.gpsimd.dma_gather(
                        out_ap=g[:, : n // 128, :], in_ap=gk[0:cfg.ASPLIT, :],
                        idxs_ap=idxA_t[:, ch0 * 8: ch0 * 8 + n // 16],
                        num_idxs=n, num_idxs_reg=n, elem_size=TOK,
                        single_packet=False)
                    GA.append(g)
                for (ch0, n) in b_calls:
                    g = gbp.tile([128, gb_free, TOK], dt.bfloat16, tag="gb")
                    nc.gpsimd.dma_gather(
                        out_ap=g[:, : n // 128, :], in_ap=gk[cfg.BBASE:, :],
                        idxs_ap=idxB_t[:, ch0 * 8: ch0 * 8 + n // 16],
                        num_idxs=n, num_idxs_reg=n, elem_size=TOK,
                        single_packet=False)
                    GB.append(g)

                for tl in range(T):
                    pt = patp.tile([128, cfg.CPT, 128], dt.bfloat16, tag="pat")
                    nc.sync.dma_start(out=pt[:], in_=patd[:].rearrange(
                        "(c s) r -> s c r", s=128)[:, tl * cfg.CPT:(tl + 1) * cfg.CPT, :])
                    ps = psp.tile([128, TOK], dt.float32, tag="ps")
                    for j in range(cfg.CPT_A):
                        ci, sl = amap[tl * cfg.CPT_A + j]
                        nc.tensor.matmul(ps[:], pt[:, j, :], GA[ci][:, sl, :],
                                         start=(j == 0), stop=False)
                    for j in range(cfg.CPT_B):
                        ci, sl = bmap[tl * cfg.CPT_B + j]
                        nc.tensor.matmul(ps[:], pt[:, cfg.CPT_A + j, :], GB[ci][:, sl, :],
                                         start=False, stop=(j == cfg.CPT_B - 1))
                    # recurrence: k=1: x1 = ps - x0 ; k>1: xk = 2 ps - 2 x_{k-1} - x_{k-2}
                    xprev = evp.tile([128, TOK], dt.float32, tag="xprev")
                    nc.sync.dma_start(out=xprev[:], in_=xf[k - 1][tl * 128:(tl + 1) * 128, :])
                    xk_t = evp.tile([128, TOK], dt.float32, tag="xk")
                    if k == 1:
                        nc.vector.scalar_tensor_tensor(
                            xk_t[:], ps[:], 1.0, xprev[:],
                            op0=mybir.AluOpType.mult, op1=mybir.AluOpType.subtract)
                    else:
                        xpp = evp.tile([128, TOK], dt.float32, tag="xpp")
                        nc.sync.dma_start(out=xpp[:], in_=xf[k - 2][tl * 128:(tl + 1) * 128, :])
                        tmp = evp.tile([128, TOK], dt.float32, tag="tmp")
                        nc.vector.scalar_tensor_tensor(
                            tmp[:], xprev[:], 2.0, xpp[:],
                            op0=mybir.AluOpType.mult, op1=mybir.AluOpType.add)
                        nc.vector.scalar_tensor_tensor(
                            xk_t[:], ps[:], 2.0, tmp[:],
                            op0=mybir.AluOpType.mult, op1=mybir.AluOpType.subtract)
                    nc.sync.dma_start(out=xf[k][tl * 128:(tl + 1) * 128, :], in_=xk_t[:])
                    xkb = evp.tile([128, TOK], dt.bfloat16, tag="xkb")
                    nc.vector.tensor_copy(xkb[:], xk_t[:])
                    nc.sync.dma_start(out=contrib[k][tl * 128:(tl + 1) * 128, :], in_=xkb[:])
                projection(k)

            for _rep in range(repeat):
                stage0()
                projection(0)
                for k in range(1, K):
                    cheb_step(k, None)

    nc.finalize()
    return nc


_NC_CACHE = {}


def get_nc(cfg, repeat=1):
    key = (cfg.M, cfg.NTILE_CORE, cfg.CPT_A, cfg.CPT_B, repeat)
    if key not in _NC_CACHE:
        _NC_CACHE[key] = build_nc(cfg, repeat)
    return _NC_CACHE[key]


# ---------------------------------------------------------------- entry
def run(cfg, x, edge_vals, W, edge_rows, edge_cols, trace=False):
    g = build_graph_data(cfg, edge_rows, edge_cols, edge_vals)
    x0 = build_x0(cfg, x, g["v2pos"])
    wb = build_w_blocks(W)
    nc = get_nc(cfg)
    in_maps = []
    for c in range(NCORE):
        in_maps.append({
            "x0loc": x0[c],
            "idxA": g["idxA_w"][c],
            "idxB": g["idxB_w"][c],
            "pat": np.ascontiguousarray(g["pat"][c].reshape(cfg.NCH * 128, 128)),
            "wblk": np.ascontiguousarray(wb.reshape(K * 128, 128)),
        })
    res = run_bass_kernel_spmd(nc, in_maps, core_ids=list(range(NCORE)), trace=trace)
    # assemble: outp [512, SLICE]; row j*128 + b_loc*64 + fo with b = 2j + b_loc
    out_all = np.stack([res.results[c]["outp"] for c in range(NCORE)], 0)  # [NC, 512, S]
    out_all = out_all.reshape(NCORE, 4, 2, FOUT, cfg.SLICE)
    out_pos = out_all.transpose(1, 2, 0, 4, 3).reshape(B, cfg.MPAD, FOUT)
    out = np.empty((B, cfg.M, FOUT), np.float32)
    out[:] = out_pos[:, g["v2pos"], :]
    return out, res


def kernel(**inputs):
    out, _ = run(CFG_FULL, inputs["x"], inputs["edge_vals"], inputs["W"],
                 inputs["edge_rows"], inputs["edge_cols"])
    return out



# revision 5
# speedup vs baseline: 84.9034x; 84.9034x over previous
"""MeshConv (Chebyshev graph conv, K=6) Trainium2 kernel, 8 NeuronCores.

Strategy: pure batch parallelism (B=8 == n_cores).  Each core owns one batch
and runs the full Chebyshev recursion on its own [M, 64] feature block, so
there are NO collectives at all.  The SpMM uses the TensorEngine: edges are
slotted host-side into a fixed per-dst-tile chunk grid; per chunk a one-hot
[128 slots x 128 rows] pattern (built on device from compact (rloc,val)
arrays) is the stationary operand against 64-wide gathered source rows
(f32 gathers: 64 feats * 4B = 256B packets).  The dense projection
accumulates k-stripes of transposed activations and finishes with a 3-chunk
GEMM against a k-major-restacked W.  Vertices stay in natural order (no
permutation), so host prep only touches the edge arrays.
"""
import sys

sys.path.insert(0, '/opt/trn_rl_repo')

import numpy as np
import ml_dtypes

import concourse.bass as bass
import concourse.bacc as bacc
import concourse.mybir as mybir
import concourse.tile as tile_mod
from concourse.tile import TileContext
from concourse import bass2jax

# ---------------------------------------------------------------- constants
B, F, K = 8, 64, 6
NCORE = 8

# walrus in this environment accepts only 1 sync-wait per CTRL instruction:
# spread the Tile tail-drain's waits across preceding nops.
def _patched_drain_and_barrier(self, tick_clock, wait_clock):
    nop0 = self.nc.sync.nop(nofuse=True)
    wait_clock.add_sem_waits(nop0.ins, tile_mod.ScopedClock({None: tick_clock.global_clock}))
    si = nop0.ins.sync_info
    waits = list(si.on_wait) if si and si.on_wait else []
    if len(waits) > 1:
        si.on_wait = waits[:1]
        rest = waits[1:]
        while rest:
            n = self.nc.sync.nop(nofuse=True)
            nsi = n.ins.sync_info
            if nsi is None:
                n.ins.sync_info = mybir.SyncInfo(on_wait=rest[:1], on_update=[])
            else:
                nsi.on_wait = rest[:1]
            rest = rest[1:]
    self.nc.sync.drain()
    self.nc.all_engine_barrier()
    assert self.sems is not None
    popped = self.nc._tile_sem_poison_stack.pop()
    assert popped is self._sem_poison
    self.nc.clear_and_free_semaphores(list(self.sems.allocated().values()))
    self.nc.all_engine_barrier()


tile_mod.TileContext._drain_and_barrier = _patched_drain_and_barrier


class Cfg:
    def __init__(self, M, mpad, asplit, bbase, cpt_a, cpt_b, ga_call, gb_call, G):
        self.M = M
        self.MPAD = mpad
        self.ASPLIT = asplit          # A gathers read rows [0, ASPLIT)
        self.BBASE = bbase            # B gathers read rows [BBASE, MPAD)
        assert asplit <= 32768 and mpad - bbase <= 32768
        self.CPT_A, self.CPT_B = cpt_a, cpt_b
        self.CPT = cpt_a + cpt_b
        self.NT = mpad // 128
        assert mpad % 128 == 0
        self.NCH = self.NT * self.CPT
        self.NIDX_A = self.NT * cpt_a * 128
        self.NIDX_B = self.NT * cpt_b * 128
        self.GA_CALL, self.GB_CALL = ga_call, gb_call
        assert self.NIDX_A % ga_call == 0 and self.NIDX_B % gb_call == 0
        self.G = G                    # dst tiles per group
        assert self.NT % G == 0
        self.NGRP = self.NT // G


CFG_FULL = Cfg(M=40000, mpad=40960, asplit=32768, bbase=8192,
               cpt_a=7, cpt_b=2, ga_call=4096, gb_call=4096, G=4)


# ---------------------------------------------------------------- host prep
def prep_graph(cfg, edge_rows, edge_cols, edge_vals):
    """Slot the edge list into the per-tile chunk grid (vectorized).

    Returns wrapped int16 gather indices and the compact pattern arrays
    (per-slot dst-row and value, [128 lanes, NCH], bf16).
    """
    er = np.asarray(edge_rows).astype(np.int64)
    ec = np.asarray(edge_cols).astype(np.int64)
    ev = np.asarray(edge_vals).astype(np.float32)
    E = er.shape[0]
    capA, capB = cfg.CPT_A * 128, cfg.CPT_B * 128

    tile = er >> 7
    cat = np.where(ec >= cfg.ASPLIT, 2, np.where(ec >= cfg.BBASE, 1, 0))
    order = np.argsort((tile << 34) | (cat.astype(np.int64) << 32) | ec, kind="stable")
    tile_s = tile[order]
    ec_s = ec[order]
    ev_s = ev[order]
    rloc_s = er[order] & 127

    n_t = np.bincount(tile_s, minlength=cfg.NT)
    bonly_t = np.bincount(tile[cat == 2], minlength=cfg.NT)
    aonly_t = np.bincount(tile[cat == 0], minlength=cfg.NT)
    needB = np.maximum(bonly_t, n_t - capA)
    nA_t = n_t - needB
    if not ((nA_t <= capA).all() and (needB <= capB).all()
            and (aonly_t <= nA_t).all() and (nA_t >= 0).all()):
        raise RuntimeError("tile slot grid overflow for this edge list")

    cum = np.zeros(cfg.NT + 1, np.int64)
    np.cumsum(n_t, out=cum[1:])
    pos = np.arange(E, dtype=np.int64) - cum[tile_s]
    isA = pos < nA_t[tile_s]

    idxA = np.zeros(cfg.NIDX_A, np.int16)
    idxB = np.zeros(cfg.NIDX_B, np.int16)
    prloc = np.zeros((128, cfg.NCH), np.float32)
    pval = np.zeros((128, cfg.NCH), np.float32)

    sA = pos[isA]
    tA = tile_s[isA]
    laneA = (sA & 127).astype(np.int64)
    idxA[(tA * cfg.CPT_A + (sA >> 7)) * 128 + laneA] = ec_s[isA].astype(np.int16)
    gchA = tA * cfg.CPT + (sA >> 7)
    prloc[laneA, gchA] = rloc_s[isA]
    pval[laneA, gchA] = ev_s[isA]

    nb = ~isA
    sB = (pos - nA_t[tile_s])[nb]
    tB = tile_s[nb]
    laneB = (sB & 127).astype(np.int64)
    idxB[(tB * cfg.CPT_B + (sB >> 7)) * 128 + laneB] = (ec_s[nb] - cfg.BBASE).astype(np.int16)
    gchB = tB * cfg.CPT + cfg.CPT_A + (sB >> 7)
    prloc[laneB, gchB] = rloc_s[nb]
    pval[laneB, gchB] = ev_s[nb]

    return {
        "idxA": np.ascontiguousarray(idxA.reshape(-1, 16).T),   # [16, NIDX_A/16]
        "idxB": np.ascontiguousarray(idxB.reshape(-1, 16).T),
        "prloc": prloc,
        "pval": pval,
    }


def prep_w(W):
    """W [F*K, F] (rows fin*K + k) -> k-major stack [K*F, F] (rows k*F + fin)."""
    Wk = np.asarray(W).astype(np.float32).reshape(F, K, F).transpose(1, 0, 2)
    return np.ascontiguousarray(Wk.reshape(K * F, F)).astype(ml_dtypes.bfloat16)


# ---------------------------------------------------------------- device IR
def build_nc(cfg, repeat=1):
    nc = bacc.Bacc(None, target_bir_lowering=False, debug=False,
                   dynamic_dma_scratch_size=16384)
    dt = mybir.dt
    G = cfg.G
    aluop = mybir.AluOpType

    xb = nc.declare_dram_parameter("xb", [cfg.M, F], dt.bfloat16, isOutput=False)
    idxA_d = nc.declare_dram_parameter("idxA", [16, cfg.NIDX_A // 16], dt.int16, isOutput=False)
    idxB_d = nc.declare_dram_parameter("idxB", [16, cfg.NIDX_B // 16], dt.int16, isOutput=False)
    prloc_d = nc.declare_dram_parameter("prloc", [128, cfg.NCH], dt.float32, isOutput=False)
    pval_d = nc.declare_dram_parameter("pval", [128, cfg.NCH], dt.float32, isOutput=False)
    wst_d = nc.declare_dram_parameter("wst", [K * F, F], dt.bfloat16, isOutput=False)
    out_d = nc.declare_dram_parameter("out", [cfg.MPAD, F], dt.bfloat16, isOutput=True)

    xs = [nc.dram_tensor(f"xs{k}", [cfg.MPAD, F], dt.float32) for k in range(K - 1)]
    xT_d = nc.dram_tensor("xT", [K * F, cfg.MPAD], dt.bfloat16)
    patd = nc.dram_tensor("patd", [cfg.NCH * 128, 128], dt.bfloat16)

    CPG_A = cfg.GA_CALL // 128       # chunks per A gather call
    CPG_B = cfg.GB_CALL // 128
    NCALL_A = cfg.NIDX_A // cfg.GA_CALL
    NCALL_B = cfg.NIDX_B // cfg.GB_CALL
    PB = 32                          # pattern chunks built per DMA batch
    NG0 = cfg.MPAD // (128 * G)      # stage0 groups

    with TileContext(nc) as tc:
        with (
            tc.tile_pool(name="io", bufs=1) as io,
            tc.tile_pool(name="patp", bufs=2) as patp,
            tc.tile_pool(name="ga", bufs=2) as gap,
            tc.tile_pool(name="gb", bufs=2) as gbp,
            tc.tile_pool(name="ev", bufs=2) as evp,
            tc.tile_pool(name="prj", bufs=2) as prjp,
            tc.tile_pool(name="ps", bufs=3, space="PSUM") as psp,
            tc.tile_pool(name="psT", bufs=2, space="PSUM") as psTp,
            tc.tile_pool(name="psg", bufs=2, space="PSUM") as psgp,
        ):
            # ---- resident tiles
            idxA_t = io.tile([128, cfg.NIDX_A // 16], dt.int16)
            idxB_t = io.tile([128, cfg.NIDX_B // 16], dt.int16)
            prlocT = io.tile([128, cfg.NCH], dt.float32)
            pvalT = io.tile([128, cfg.NCH], dt.float32)
            wsb = io.tile([128, K * F // 128, F], dt.bfloat16)
            iota_i = io.tile([128, 128], dt.int16)
            iota_b = io.tile([128, 128], dt.float32)
            pcol_i = io.tile([128, 1], dt.int16)
            pcol_b = io.tile([128, 1], dt.float32)
            ident_t = io.tile([128, 128], dt.bfloat16)

            for i in range(8):
                nc.sync.dma_start(out=idxA_t[16 * i:16 * (i + 1), :], in_=idxA_d[:])
                nc.sync.dma_start(out=idxB_t[16 * i:16 * (i + 1), :], in_=idxB_d[:])
            nc.sync.dma_start(out=prlocT[:], in_=prloc_d[:])
            nc.sync.dma_start(out=pvalT[:], in_=pval_d[:])
            nc.sync.dma_start(out=wsb[:], in_=wst_d[:].rearrange("(j p) f -> p j f", p=128))
            nc.gpsimd.iota(iota_i[:], pattern=[[1, 128]], base=0, channel_multiplier=0)
            nc.vector.tensor_copy(iota_b[:], iota_i[:])
            nc.gpsimd.iota(pcol_i[:], pattern=[[0, 1]], base=0, channel_multiplier=1)
            nc.vector.tensor_copy(pcol_b[:], pcol_i[:])
            nc.vector.tensor_scalar(ident_t[:], iota_b[:], pcol_b[:, 0:1], None,
                                    op0=aluop.is_equal)

            patd_v = patd[:].rearrange("(c p) r -> p c r", p=128)

            def body():
                # ---- pattern build: pat[lane, r] = (r == rloc[lane]) * val[lane]
                for c0 in range(0, cfg.NCH, PB):
                    nchb = min(PB, cfg.NCH - c0)
                    pt = patp.tile([128, PB, 128], dt.bfloat16, tag="pb")
                    for j in range(nchb):
                        nc.vector.tensor_scalar(
                            pt[:, j, :], iota_b[:], prlocT[:, c0 + j:c0 + j + 1],
                            pvalT[:, c0 + j:c0 + j + 1],
                            op0=aluop.is_equal, op1=aluop.mult)
                    nc.sync.dma_start(out=patd_v[:, c0:c0 + nchb, :], in_=pt[:, :nchb, :])

                # ---- stage0: xb -> xs[0] (f32, zero-padded) + xT stripe 0
                for g in range(NG0):
                    r0 = g * 128 * G
                    nreal = min(max(cfg.M - r0, 0), 128 * G)
                    t0 = evp.tile([128, G, F], dt.bfloat16, tag="t0")
                    if nreal < 128 * G:
                        nc.vector.memset(t0[:], 0.0)
                    ft = nreal // 128
                    if ft:
                        nc.sync.dma_start(
                            out=t0[:, :ft, :],
                            in_=xb[r0:r0 + 128 * ft, :].rearrange("(a p) f -> p a f", p=128))
                    rem = nreal % 128
                    if rem:
                        nc.sync.dma_start(out=t0[:rem, ft, :],
                                          in_=xb[r0 + 128 * ft:r0 + nreal, :])
                    t0f = evp.tile([128, G, F], dt.float32, tag="t0f")
                    nc.vector.tensor_copy(t0f[:], t0[:])
                    nc.sync.dma_start(
                        out=xs[0][r0:r0 + 128 * G, :].rearrange("(a p) f -> p a f", p=128),
                        in_=t0f[:])
                    tp = psTp.tile([64, G, 128], dt.bfloat16, tag="tp")
                    for t in range(G):
                        nc.tensor.transpose(tp[:, t, :], t0[:, t, :], ident_t[:])
                    tps = evp.tile([64, G, 128], dt.bfloat16, tag="tps")
                    nc.vector.tensor_copy(tps[:], tp[:])
                    nc.sync.dma_start(out=xT_d[0:F, r0:r0 + 128 * G], in_=tps[:])

                # ---- Chebyshev steps
                for k in range(1, K):
                    src = xs[k - 1]
                    GAB, GBB = [], []
                    for ci in range(NCALL_A):
                        gt = gap.tile([128, CPG_A, F], dt.float32, tag="ga")
                        nc.gpsimd.dma_gather(
                            out_ap=gt[:], in_ap=src[0:cfg.ASPLIT, :],
                            idxs_ap=idxA_t[:, ci * (cfg.GA_CALL // 16):(ci + 1) * (cfg.GA_CALL // 16)],
                            num_idxs=cfg.GA_CALL, num_idxs_reg=cfg.GA_CALL,
                            elem_size=F, single_packet=False)
                        gtb = gap.tile([128, CPG_A, F], dt.bfloat16, tag="gab")
                        nc.vector.tensor_copy(gtb[:], gt[:])
                        GAB.append(gtb)
                    for ci in range(NCALL_B):
                        gt = gbp.tile([128, CPG_B, F], dt.float32, tag="gb")
                        nc.gpsimd.dma_gather(
                            out_ap=gt[:], in_ap=src[cfg.BBASE:cfg.MPAD, :],
                            idxs_ap=idxB_t[:, ci * (cfg.GB_CALL // 16):(ci + 1) * (cfg.GB_CALL // 16)],
                            num_idxs=cfg.GB_CALL, num_idxs_reg=cfg.GB_CALL,
                            elem_size=F, single_packet=False)
                        gtb = gbp.tile([128, CPG_B, F], dt.bfloat16, tag="gbb")
                        nc.vector.tensor_copy(gtb[:], gt[:])
                        GBB.append(gtb)

                    for grp in range(cfg.NGRP):
                        r0 = grp * 128 * G
                        pt = patp.tile([128, G * cfg.CPT, 128], dt.bfloat16, tag="pat")
                        nc.sync.dma_start(
                            out=pt[:], in_=patd_v[:, grp * G * cfg.CPT:(grp + 1) * G * cfg.CPT, :])
                        ps = psp.tile([128, G, F], dt.float32, tag="ps")
                        for t in range(G):
                            tid = grp * G + t
                            for j in range(cfg.CPT):
                                if j < cfg.CPT_A:
                                    ca = tid * cfg.CPT_A + j
                                    mov = GAB[ca // CPG_A][:, ca % CPG_A, :]
                                else:
                                    cb = tid * cfg.CPT_B + (j - cfg.CPT_A)
                                    mov = GBB[cb // CPG_B][:, cb % CPG_B, :]
                                nc.tensor.matmul(ps[:, t, :], pt[:, t * cfg.CPT + j, :], mov,
                                                 start=(j == 0), stop=(j == cfg.CPT - 1))
                        xc = evp.tile([128, G, F], dt.float32, tag="xc")
                        nc.sync.dma_start(
                            out=xc[:],
                            in_=src[r0:r0 + 128 * G, :].rearrange("(a p) f -> p a f", p=128))
                        xk_t = evp.tile([128, G, F], dt.float32, tag="xk")
                        if k == 1:
                            nc.vector.tensor_sub(xk_t[:], ps[:], xc[:])
                        else:
                            xp = evp.tile([128, G, F], dt.float32, tag="xp")
                            nc.sync.dma_start(
                                out=xp[:],
                                in_=xs[k - 2][r0:r0 + 128 * G, :].rearrange("(a p) f -> p a f", p=128))
                            tmp = evp.tile([128, G, F], dt.float32, tag="tmp")
                            nc.vector.tensor_sub(tmp[:], ps[:], xc[:])
                            nc.vector.scalar_tensor_tensor(
                                xk_t[:], tmp[:], 2.0, xp[:],
                                op0=aluop.mult, op1=aluop.subtract)
                        if k < K - 1:
                            nc.sync.dma_start(
                                out=xs[k][r0:r0 + 128 * G, :].rearrange("(a p) f -> p a f", p=128),
                                in_=xk_t[:])
                        xkb = evp.tile([128, G, F], dt.bfloat16, tag="xkb")
                        nc.vector.tensor_copy(xkb[:], xk_t[:])
                        tp = psTp.tile([64, G, 128], dt.bfloat16, tag="tp")
                        for t in range(G):
                            nc.tensor.transpose(tp[:, t, :], xkb[:, t, :], ident_t[:])
                        tps = evp.tile([64, G, 128], dt.bfloat16, tag="tps")
                        nc.vector.tensor_copy(tps[:], tp[:])
                        nc.sync.dma_start(out=xT_d[k * F:(k + 1) * F, r0:r0 + 128 * G],
                                          in_=tps[:])

                # ---- dense projection: out = X_cat @ W  (3 stat chunks of 128)
                for grp in range(cfg.NGRP):
                    r0 = grp * 128 * G
                    stx = prjp.tile([128, 3, G * 128], dt.bfloat16, tag="stx")
                    for j in range(3):
                        nc.sync.dma_start(out=stx[:, j, :],
                                          in_=xT_d[128 * j:128 * (j + 1), r0:r0 + 128 * G])
                    pg = psgp.tile([128, G, F], dt.float32, tag="pg")
                    for t in range(G):
                        for j in range(3):
                            nc.tensor.matmul(pg[:, t, :], stx[:, j, t * 128:(t + 1) * 128],
                                             wsb[:, j, :], start=(j == 0), stop=(j == 2))
                    ob = prjp.tile([128, G, F], dt.bfloat16, tag="ob")
                    nc.vector.tensor_copy(ob[:], pg[:])
                    nc.sync.dma_start(
                        out=out_d[r0:r0 + 128 * G, :].rearrange("(a p) f -> p a f", p=128),
                        in_=ob[:])

            for _rep in range(repeat):
                body()

    nc.finalize()
    return nc


_NC_CACHE = {}


def get_nc(cfg, repeat=1):
    key = (cfg.M, cfg.MPAD, cfg.CPT_A, cfg.CPT_B, cfg.G, repeat)
    if key not in _NC_CACHE:
        _NC_CACHE[key] = build_nc(cfg, repeat)
    return _NC_CACHE[key]


# ---------------------------------------------------------------- entry
def make_in_maps(cfg, x, edge_vals, W, edge_rows, edge_cols):
    g = prep_graph(cfg, edge_rows, edge_cols, edge_vals)
    wst = prep_w(W)
    xb16 = np.asarray(x).astype(ml_dtypes.bfloat16)
    in_maps = []
    for c in range(NCORE):
        in_maps.append({
            "xb": xb16[c],
            "idxA": g["idxA"],
            "idxB": g["idxB"],
            "prloc": g["prloc"],
            "pval": g["pval"],
            "wst": wst,
        })
    return in_maps


def run(cfg, x, edge_vals, W, edge_rows, edge_cols):
    in_maps = make_in_maps(cfg, x, edge_vals, W, edge_rows, edge_cols)
    nc = get_nc(cfg)
    results = bass2jax.run_bass_via_pjrt(nc, in_maps, n_cores=NCORE)
    Bd = np.asarray(x).shape[0]
    out = np.empty((Bd, cfg.M, F), np.float32)
    for c in range(Bd):
        out[c] = results[c]["out"][:cfg.M].astype(np.float32)
    return out


def kernel(**inputs):
    return run(CFG_FULL, inputs["x"], inputs["edge_vals"], inputs["W"],
               inputs["edge_rows"], inputs["edge_cols"])


# revision 9
# speedup vs baseline: 99.9906x; 1.1777x over previous
"""MeshConv (Chebyshev graph conv, K=6) Trainium2 kernel, 8 NeuronCores.

Strategy: pure batch parallelism (B=8 == n_cores).  Each core owns one batch
and runs the full Chebyshev recursion on its own [M, 64] feature block, so
there are NO collectives at all.  The SpMM uses the TensorEngine: edges are
slotted host-side into a fixed per-dst-tile chunk grid; per chunk a one-hot
[128 slots x 128 rows] pattern (built on device from compact (rloc,val)
arrays) is the stationary operand against 64-wide gathered source rows
(f32 gathers: 64 feats * 4B = 256B packets).  The dense projection
accumulates k-stripes of transposed activations and finishes with a 3-chunk
GEMM against a k-major-restacked W.  Vertices stay in natural order (no
permutation), so host prep only touches the edge arrays.
"""
import sys

sys.path.insert(0, '/opt/trn_rl_repo')

import numpy as np
import ml_dtypes

import concourse.bass as bass
import concourse.bacc as bacc
import concourse.mybir as mybir
import concourse.tile as tile_mod
from concourse.tile import TileContext
from concourse import bass2jax

# ---------------------------------------------------------------- constants
B, F, K = 8, 64, 6
NCORE = 8

# walrus in this environment accepts only 1 sync-wait per CTRL instruction:
# spread the Tile tail-drain's waits across preceding nops.
def _patched_drain_and_barrier(self, tick_clock, wait_clock):
    nop0 = self.nc.sync.nop(nofuse=True)
    wait_clock.add_sem_waits(nop0.ins, tile_mod.ScopedClock({None: tick_clock.global_clock}))
    si = nop0.ins.sync_info
    waits = list(si.on_wait) if si and si.on_wait else []
    if len(waits) > 1:
        si.on_wait = waits[:1]
        rest = waits[1:]
        while rest:
            n = self.nc.sync.nop(nofuse=True)
            nsi = n.ins.sync_info
            if nsi is None:
                n.ins.sync_info = mybir.SyncInfo(on_wait=rest[:1], on_update=[])
            else:
                nsi.on_wait = rest[:1]
            rest = rest[1:]
    self.nc.sync.drain()
    self.nc.all_engine_barrier()
    assert self.sems is not None
    popped = self.nc._tile_sem_poison_stack.pop()
    assert popped is self._sem_poison
    self.nc.clear_and_free_semaphores(list(self.sems.allocated().values()))
    self.nc.all_engine_barrier()


tile_mod.TileContext._drain_and_barrier = _patched_drain_and_barrier


class Cfg:
    def __init__(self, M, mpad, asplit, bbase, cpt_a, cpt_b, ga_call, gb_call, G):
        self.M = M
        self.MPAD = mpad
        self.ASPLIT = asplit          # A gathers read rows [0, ASPLIT)
        self.BBASE = bbase            # B gathers read rows [BBASE, MPAD)
        assert asplit <= 32768 and mpad - bbase <= 32768
        self.CPT_A, self.CPT_B = cpt_a, cpt_b
        self.CPT = cpt_a + cpt_b
        self.NT = mpad // 128
        assert mpad % 128 == 0
        self.NCH = self.NT * self.CPT
        self.NIDX_A = self.NT * cpt_a * 128
        self.NIDX_B = self.NT * cpt_b * 128
        self.GA_CALL, self.GB_CALL = ga_call, gb_call
        assert self.NIDX_A % ga_call == 0 and self.NIDX_B % gb_call == 0
        self.G = G                    # dst tiles per group
        assert self.NT % G == 0
        self.NGRP = self.NT // G


CFG_FULL = Cfg(M=40000, mpad=40960, asplit=32768, bbase=8192,
               cpt_a=7, cpt_b=2, ga_call=4096, gb_call=4096, G=4)


# ---------------------------------------------------------------- host prep
def prep_graph(cfg, edge_rows, edge_cols, edge_vals):
    """Slot the edge list into the per-tile chunk grid (vectorized).

    Returns wrapped int16 gather indices and the compact pattern arrays
    (per-slot dst-row and value, [128 lanes, NCH], bf16).
    """
    er = np.asarray(edge_rows).astype(np.int64)
    ec = np.asarray(edge_cols).astype(np.int64)
    ev = np.asarray(edge_vals).astype(np.float32)
    E = er.shape[0]
    capA, capB = cfg.CPT_A * 128, cfg.CPT_B * 128

    tile = er >> 7
    cat = np.where(ec >= cfg.ASPLIT, 2, np.where(ec >= cfg.BBASE, 1, 0))
    order = np.argsort((tile << 34) | (cat.astype(np.int64) << 32) | ec, kind="stable")
    tile_s = tile[order]
    ec_s = ec[order]
    ev_s = ev[order]
    rloc_s = er[order] & 127

    n_t = np.bincount(tile_s, minlength=cfg.NT)
    bonly_t = np.bincount(tile[cat == 2], minlength=cfg.NT)
    aonly_t = np.bincount(tile[cat == 0], minlength=cfg.NT)
    needB = np.maximum(bonly_t, n_t - capA)
    nA_t = n_t - needB
    if not ((nA_t <= capA).all() and (needB <= capB).all()
            and (aonly_t <= nA_t).all() and (nA_t >= 0).all()):
        raise RuntimeError("tile slot grid overflow for this edge list")

    cum = np.zeros(cfg.NT + 1, np.int64)
    np.cumsum(n_t, out=cum[1:])
    pos = np.arange(E, dtype=np.int64) - cum[tile_s]
    isA = pos < nA_t[tile_s]

    idxA = np.zeros(cfg.NIDX_A, np.int16)
    idxB = np.zeros(cfg.NIDX_B, np.int16)
    prloc = np.zeros((128, cfg.NCH), np.float32)
    pval = np.zeros((128, cfg.NCH), np.float32)

    sA = pos[isA]
    tA = tile_s[isA]
    laneA = (sA & 127).astype(np.int64)
    idxA[(tA * cfg.CPT_A + (sA >> 7)) * 128 + laneA] = ec_s[isA].astype(np.int16)
    gchA = tA * cfg.CPT + (sA >> 7)
    prloc[laneA, gchA] = rloc_s[isA]
    pval[laneA, gchA] = ev_s[isA]

    nb = ~isA
    sB = (pos - nA_t[tile_s])[nb]
    tB = tile_s[nb]
    laneB = (sB & 127).astype(np.int64)
    idxB[(tB * cfg.CPT_B + (sB >> 7)) * 128 + laneB] = (ec_s[nb] - cfg.BBASE).astype(np.int16)
    gchB = tB * cfg.CPT + cfg.CPT_A + (sB >> 7)
    prloc[laneB, gchB] = rloc_s[nb]
    pval[laneB, gchB] = ev_s[nb]

    return {
        "idxA": np.ascontiguousarray(idxA.reshape(-1, 16).T),   # [16, NIDX_A/16]
        "idxB": np.ascontiguousarray(idxB.reshape(-1, 16).T),
        "prloc": prloc,
        "pval": pval,
    }


def prep_w(W):
    """W [F*K, F] (rows fin*K + k) -> k-major stack [K*F, F] (rows k*F + fin)."""
    Wk = np.asarray(W).astype(np.float32).reshape(F, K, F).transpose(1, 0, 2)
    return np.ascontiguousarray(Wk.reshape(K * F, F)).astype(ml_dtypes.bfloat16)


# ---------------------------------------------------------------- device IR
def build_nc(cfg, repeat=1, ablate=(), nq=4):
    nc = bacc.Bacc(None, target_bir_lowering=False, debug=False,
                   dynamic_dma_scratch_size=16384, num_swdge_queues=nq)
    dt = mybir.dt
    G = cfg.G
    aluop = mybir.AluOpType

    xb = nc.declare_dram_parameter("xb", [cfg.M, F], dt.bfloat16, isOutput=False)
    idxA_d = nc.declare_dram_parameter("idxA", [16, cfg.NIDX_A // 16], dt.int16, isOutput=False)
    idxB_d = nc.declare_dram_parameter("idxB", [16, cfg.NIDX_B // 16], dt.int16, isOutput=False)
    prloc_d = nc.declare_dram_parameter("prloc", [128, cfg.NCH], dt.float32, isOutput=False)
    pval_d = nc.declare_dram_parameter("pval", [128, cfg.NCH], dt.float32, isOutput=False)
    wst_d = nc.declare_dram_parameter("wst", [K * F, F], dt.bfloat16, isOutput=False)
    out_d = nc.declare_dram_parameter("out", [cfg.MPAD, F], dt.bfloat16, isOutput=True)

    xs = [nc.dram_tensor(f"xs{k}", [cfg.MPAD, F], dt.float32) for k in range(K - 1)]
    xT_d = nc.dram_tensor("xT", [K * F, cfg.MPAD], dt.bfloat16)
    patd = nc.dram_tensor("patd", [cfg.NCH * 128, 128], dt.bfloat16)

    CPG_A = cfg.GA_CALL // 128       # chunks per A gather call
    CPG_B = cfg.GB_CALL // 128
    NCALL_A = cfg.NIDX_A // cfg.GA_CALL
    NCALL_B = cfg.NIDX_B // cfg.GB_CALL
    PB = 32                          # pattern chunks built per DMA batch
    NG0 = cfg.MPAD // (128 * G)      # stage0 groups

    with TileContext(nc) as tc:
        with (
            tc.tile_pool(name="io", bufs=1) as io,
            tc.tile_pool(name="patp", bufs=2) as patp,
            tc.tile_pool(name="ga", bufs=2) as gap,
            tc.tile_pool(name="gb", bufs=2) as gbp,
            tc.tile_pool(name="ev", bufs=2) as evp,
            tc.tile_pool(name="prj", bufs=2) as prjp,
            tc.tile_pool(name="ps", bufs=3, space="PSUM") as psp,
            tc.tile_pool(name="psT", bufs=2, space="PSUM") as psTp,
            tc.tile_pool(name="psg", bufs=2, space="PSUM") as psgp,
        ):
            # ---- resident tiles
            idxA_t = io.tile([128, cfg.NIDX_A // 16], dt.int16)
            idxB_t = io.tile([128, cfg.NIDX_B // 16], dt.int16)
            prlocT = io.tile([128, cfg.NCH], dt.float32)
            pvalT = io.tile([128, cfg.NCH], dt.float32)
            wsb = io.tile([128, K * F // 128, F], dt.bfloat16)
            iota_i = io.tile([128, 128], dt.int16)
            iota_b = io.tile([128, 128], dt.float32)
            pcol_i = io.tile([128, 1], dt.int16)
            pcol_b = io.tile([128, 1], dt.float32)
            ident_t = io.tile([128, 128], dt.bfloat16)

            for i in range(8):
                nc.sync.dma_start(out=idxA_t[16 * i:16 * (i + 1), :], in_=idxA_d[:])
                nc.sync.dma_start(out=idxB_t[16 * i:16 * (i + 1), :], in_=idxB_d[:])
            nc.sync.dma_start(out=prlocT[:], in_=prloc_d[:])
            nc.sync.dma_start(out=pvalT[:], in_=pval_d[:])
            nc.sync.dma_start(out=wsb[:], in_=wst_d[:].rearrange("(j p) f -> p j f", p=128))
            nc.gpsimd.iota(iota_i[:], pattern=[[1, 128]], base=0, channel_multiplier=0)
            nc.vector.tensor_copy(iota_b[:], iota_i[:])
            nc.gpsimd.iota(pcol_i[:], pattern=[[0, 1]], base=0, channel_multiplier=1)
            nc.vector.tensor_copy(pcol_b[:], pcol_i[:])
            nc.vector.tensor_scalar(ident_t[:], iota_b[:], pcol_b[:, 0:1], None,
                                    op0=aluop.is_equal)

            patd_v = patd[:].rearrange("(c p) r -> p c r", p=128)

            gshA = gshB = pshared = None
            if "patdma" in ablate:
                pshared = io.tile([128, cfg.G * cfg.CPT, 128], dt.bfloat16)
                nc.vector.memset(pshared[:], 0.0)
            if "gather" in ablate or "gcast" in ablate:
                gshA = io.tile([128, CPG_A, F], dt.bfloat16)
                nc.vector.memset(gshA[:], 0.0)
                gshB = io.tile([128, CPG_B, F], dt.bfloat16)
                nc.vector.memset(gshB[:], 0.0)

            def body():
                # ---- pattern build: pat[lane, r] = (r == rloc[lane]) * val[lane]
                for c0 in (() if "patbuild" in ablate else range(0, cfg.NCH, PB)):
                    nchb = min(PB, cfg.NCH - c0)
                    pt = patp.tile([128, PB, 128], dt.bfloat16, tag="pb")
                    for j in range(nchb):
                        nc.vector.tensor_scalar(
                            pt[:, j, :], iota_b[:], prlocT[:, c0 + j:c0 + j + 1],
                            pvalT[:, c0 + j:c0 + j + 1],
                            op0=aluop.is_equal, op1=aluop.mult)
                    nc.sync.dma_start(out=patd_v[:, c0:c0 + nchb, :], in_=pt[:, :nchb, :])

                # ---- stage0: xb -> xs[0] (f32, zero-padded) + xT stripe 0
                for g in range(NG0):
                    r0 = g * 128 * G
                    nreal = min(max(cfg.M - r0, 0), 128 * G)
                    t0 = evp.tile([128, G, F], dt.bfloat16, tag="t0")
                    if nreal < 128 * G:
                        nc.vector.memset(t0[:], 0.0)
                    ft = nreal // 128
                    if ft:
                        nc.sync.dma_start(
                            out=t0[:, :ft, :],
                            in_=xb[r0:r0 + 128 * ft, :].rearrange("(a p) f -> p a f", p=128))
                    rem = nreal % 128
                    if rem:
                        nc.sync.dma_start(out=t0[:rem, ft, :],
                                          in_=xb[r0 + 128 * ft:r0 + nreal, :])
                    t0f = evp.tile([128, G, F], dt.float32, tag="t0f")
                    nc.vector.tensor_copy(t0f[:], t0[:])
                    nc.sync.dma_start(
                        out=xs[0][r0:r0 + 128 * G, :].rearrange("(a p) f -> p a f", p=128),
                        in_=t0f[:])
                    tp = psTp.tile([64, G, 128], dt.bfloat16, tag="tp")
                    for t in range(G):
                        nc.tensor.transpose(tp[:, t, :], t0[:, t, :], ident_t[:])
                    tps = evp.tile([64, G, 128], dt.bfloat16, tag="tps")
                    nc.vector.tensor_copy(tps[:], tp[:])
                    nc.sync.dma_start(out=xT_d[0:F, r0:r0 + 128 * G], in_=tps[:])

                # ---- Chebyshev steps
                for k in range(1, K):
                    src = xs[k - 1]
                    GAB, GBB = [], []
                    if "gather" in ablate:
                        GAB = [gshA] * NCALL_A
                        GBB = [gshB] * NCALL_B
                    else:
                        for ci in range(NCALL_A):
                            gt = gap.tile([128, CPG_A, F], dt.float32, tag="ga")
                            nc.gpsimd.dma_gather(
                                out_ap=gt[:], in_ap=src[0:cfg.ASPLIT, :],
                                idxs_ap=idxA_t[:, ci * (cfg.GA_CALL // 16):(ci + 1) * (cfg.GA_CALL // 16)],
                                num_idxs=cfg.GA_CALL, num_idxs_reg=cfg.GA_CALL,
                                elem_size=F, single_packet=False, queue_num=ci % nq)
                            if "gcast" in ablate:
                                GAB.append(gshA)
                                continue
                            gtb = gap.tile([128, CPG_A, F], dt.bfloat16, tag="gab")
                            nc.vector.tensor_copy(gtb[:], gt[:])
                            GAB.append(gtb)
                        for ci in range(NCALL_B):
                            gt = gbp.tile([128, CPG_B, F], dt.float32, tag="gb")
                            nc.gpsimd.dma_gather(
                                out_ap=gt[:], in_ap=src[cfg.BBASE:cfg.MPAD, :],
                                idxs_ap=idxB_t[:, ci * (cfg.GB_CALL // 16):(ci + 1) * (cfg.GB_CALL // 16)],
                                num_idxs=cfg.GB_CALL, num_idxs_reg=cfg.GB_CALL,
                                elem_size=F, single_packet=False, queue_num=(ci + 1) % nq)
                            if "gcast" in ablate:
                                GBB.append(gshB)
                                continue
                            gtb = gbp.tile([128, CPG_B, F], dt.bfloat16, tag="gbb")
                            nc.vector.tensor_copy(gtb[:], gt[:])
                            GBB.append(gtb)

                    for grp in range(cfg.NGRP):
                        r0 = grp * 128 * G
                        if "patdma" in ablate:
                            pt = pshared
                        else:
                            pt = patp.tile([128, G * cfg.CPT, 128], dt.bfloat16, tag="pat")
                            nc.sync.dma_start(
                                out=pt[:], in_=patd_v[:, grp * G * cfg.CPT:(grp + 1) * G * cfg.CPT, :])
                        ps = psp.tile([128, G, F], dt.float32, tag="ps")
                        if "mm" in ablate:
                            nc.vector.memset(ps[:], 0.0)
                        else:
                            for t in range(G):
                                tid = grp * G + t
                                for j in range(cfg.CPT):
                                    if j < cfg.CPT_A:
                                        ca = tid * cfg.CPT_A + j
                                        mov = GAB[ca // CPG_A][:, ca % CPG_A, :]
                                    else:
                                        cb = tid * cfg.CPT_B + (j - cfg.CPT_A)
                                        mov = GBB[cb // CPG_B][:, cb % CPG_B, :]
                                    nc.tensor.matmul(ps[:, t, :], pt[:, t * cfg.CPT + j, :], mov,
                                                     start=(j == 0), stop=(j == cfg.CPT - 1))
                        xc = evp.tile([128, G, F], dt.float32, tag="xc")
                        nc.sync.dma_start(
                            out=xc[:],
                            in_=src[r0:r0 + 128 * G, :].rearrange("(a p) f -> p a f", p=128))
                        xk_t = evp.tile([128, G, F], dt.float32, tag="xk")
                        if k == 1:
                            nc.vector.tensor_sub(xk_t[:], ps[:], xc[:])
                        else:
                            xp = evp.tile([128, G, F], dt.float32, tag="xp")
                            nc.sync.dma_start(
                                out=xp[:],
                                in_=xs[k - 2][r0:r0 + 128 * G, :].rearrange("(a p) f -> p a f", p=128))
                            tmp = evp.tile([128, G, F], dt.float32, tag="tmp")
                            nc.vector.tensor_sub(tmp[:], ps[:], xc[:])
                            nc.vector.scalar_tensor_tensor(
                                xk_t[:], tmp[:], 2.0, xp[:],
                                op0=aluop.mult, op1=aluop.subtract)
                        if k < K - 1:
                            nc.sync.dma_start(
                                out=xs[k][r0:r0 + 128 * G, :].rearrange("(a p) f -> p a f", p=128),
                                in_=xk_t[:])
                        if "proj" not in ablate:
                            xkb = evp.tile([128, G, F], dt.bfloat16, tag="xkb")
                            nc.vector.tensor_copy(xkb[:], xk_t[:])
                            tp = psTp.tile([64, G, 128], dt.bfloat16, tag="tp")
                            for t in range(G):
                                nc.tensor.transpose(tp[:, t, :], xkb[:, t, :], ident_t[:])
                            tps = evp.tile([64, G, 128], dt.bfloat16, tag="tps")
                            nc.vector.tensor_copy(tps[:], tp[:])
                            nc.sync.dma_start(out=xT_d[k * F:(k + 1) * F, r0:r0 + 128 * G],
                                              in_=tps[:])

                # ---- dense projection: out = X_cat @ W  (3 stat chunks of 128)
                for grp in (() if "proj" in ablate else range(cfg.NGRP)):
                    r0 = grp * 128 * G
                    stx = prjp.tile([128, 3, G * 128], dt.bfloat16, tag="stx")
                    for j in range(3):
                        nc.sync.dma_start(out=stx[:, j, :],
                                          in_=xT_d[128 * j:128 * (j + 1), r0:r0 + 128 * G])
                    pg = psgp.tile([128, G, F], dt.float32, tag="pg")
                    for t in range(G):
                        for j in range(3):
                            nc.tensor.matmul(pg[:, t, :], stx[:, j, t * 128:(t + 1) * 128],
                                             wsb[:, j, :], start=(j == 0), stop=(j == 2))
                    ob = prjp.tile([128, G, F], dt.bfloat16, tag="ob")
                    nc.vector.tensor_copy(ob[:], pg[:])
                    nc.sync.dma_start(
                        out=out_d[r0:r0 + 128 * G, :].rearrange("(a p) f -> p a f", p=128),
                        in_=ob[:])

            for _rep in range(repeat):
                body()

    nc.finalize()
    return nc


_NC_CACHE = {}


def get_nc(cfg, repeat=1):
    key = (cfg.M, cfg.MPAD, cfg.CPT_A, cfg.CPT_B, cfg.G, repeat)
    if key not in _NC_CACHE:
        _NC_CACHE[key] = build_nc(cfg, repeat)
    return _NC_CACHE[key]


# ---------------------------------------------------------------- entry
def make_in_maps(cfg, x, edge_vals, W, edge_rows, edge_cols):
    g = prep_graph(cfg, edge_rows, edge_cols, edge_vals)
    wst = prep_w(W)
    xb16 = np.asarray(x).astype(ml_dtypes.bfloat16)
    in_maps = []
    for c in range(NCORE):
        in_maps.append({
            "xb": xb16[c],
            "idxA": g["idxA"],
            "idxB": g["idxB"],
            "prloc": g["prloc"],
            "pval": g["pval"],
            "wst": wst,
        })
    return in_maps


def run(cfg, x, edge_vals, W, edge_rows, edge_cols):
    in_maps = make_in_maps(cfg, x, edge_vals, W, edge_rows, edge_cols)
    nc = get_nc(cfg)
    results = bass2jax.run_bass_via_pjrt(nc, in_maps, n_cores=NCORE)
    Bd = np.asarray(x).shape[0]
    out = np.empty((Bd, cfg.M, F), np.float32)
    for c in range(Bd):
        out[c] = results[c]["out"][:cfg.M].astype(np.float32)
    return out


def kernel(**inputs):
    return run(CFG_FULL, inputs["x"], inputs["edge_vals"], inputs["W"],
               inputs["edge_rows"], inputs["edge_cols"])


# revision 11
# speedup vs baseline: 116.2377x; 1.1625x over previous
"""MeshConv (Chebyshev graph conv, K=6) Trainium2 kernel, 8 NeuronCores.

Strategy: pure batch parallelism (B=8 == n_cores).  Each core owns one batch
and runs the full Chebyshev recursion on its own [M, 64] feature block, so
there are NO collectives at all.  The SpMM uses the TensorEngine: edges are
slotted host-side into per-dst-tile chunks (variable count per tile, sized
to the tile's actual edge load); per chunk a one-hot [128 slots x 128 rows]
pattern (built on device from compact (rloc,val) arrays) is the stationary
operand against 64-wide gathered source rows (f32 gathers: 64 feats * 4B =
256B packets, spread over 4 SWDGE queues).  The dense projection accumulates
k-stripes of transposed activations and finishes with a 3-chunk GEMM against
a k-major-restacked W.  Vertices stay in natural order (no permutation), so
host prep only touches the edge arrays.
"""
import sys

sys.path.insert(0, '/opt/trn_rl_repo')

import numpy as np
import ml_dtypes

import concourse.bass as bass
import concourse.bacc as bacc
import concourse.mybir as mybir
import concourse.tile as tile_mod
from concourse.tile import TileContext
from concourse import bass2jax

# ---------------------------------------------------------------- constants
B, F, K = 8, 64, 6
NCORE = 8
PB = 32          # pattern chunks built per DVE/DMA batch

# walrus in this environment accepts only 1 sync-wait per CTRL instruction:
# spread the Tile tail-drain's waits across preceding nops.
def _patched_drain_and_barrier(self, tick_clock, wait_clock):
    nop0 = self.nc.sync.nop(nofuse=True)
    wait_clock.add_sem_waits(nop0.ins, tile_mod.ScopedClock({None: tick_clock.global_clock}))
    si = nop0.ins.sync_info
    waits = list(si.on_wait) if si and si.on_wait else []
    if len(waits) > 1:
        si.on_wait = waits[:1]
        rest = waits[1:]
        while rest:
            n = self.nc.sync.nop(nofuse=True)
            nsi = n.ins.sync_info
            if nsi is None:
                n.ins.sync_info = mybir.SyncInfo(on_wait=rest[:1], on_update=[])
            else:
                nsi.on_wait = rest[:1]
            rest = rest[1:]
    self.nc.sync.drain()
    self.nc.all_engine_barrier()
    assert self.sems is not None
    popped = self.nc._tile_sem_poison_stack.pop()
    assert popped is self._sem_poison
    self.nc.clear_and_free_semaphores(list(self.sems.allocated().values()))
    self.nc.all_engine_barrier()


tile_mod.TileContext._drain_and_barrier = _patched_drain_and_barrier


class Cfg:
    def __init__(self, M, mpad, asplit, bbase, ga_call, gb_call, G):
        self.M = M
        self.MPAD = mpad
        self.ASPLIT = asplit          # A gathers read rows [0, ASPLIT)
        self.BBASE = bbase            # B gathers read rows [BBASE, MPAD)
        assert asplit <= 32768 and mpad - bbase <= 32768
        self.NT = mpad // 128
        assert mpad % 128 == 0
        self.GA_CALL, self.GB_CALL = ga_call, gb_call
        self.G = G                    # dst tiles per group
        assert self.NT % G == 0
        self.NGRP = self.NT // G


CFG_FULL = Cfg(M=40000, mpad=40960, asplit=32768, bbase=8192,
               ga_call=4096, gb_call=4096, G=4)


def _rup(x, m):
    return (x + m - 1) // m * m


# ---------------------------------------------------------------- host prep
def prep_graph(cfg, edge_rows, edge_cols, edge_vals):
    """Slot the edge list into per-tile variable chunk lists (vectorized).

    Returns wrapped int16 gather indices, compact pattern arrays
    ([128 lanes, NCHV_PAD] f32 rloc/val), and the chunk-grid meta the
    device build needs (per-tile A/B chunk counts and offsets).
    """
    er = np.asarray(edge_rows).astype(np.int64)
    ec = np.asarray(edge_cols).astype(np.int64)
    ev = np.asarray(edge_vals).astype(np.float32)
    E = er.shape[0]

    tile = er >> 7
    cat = np.where(ec >= cfg.ASPLIT, 2, np.where(ec >= cfg.BBASE, 1, 0))
    order = np.argsort((tile << 34) | (cat.astype(np.int64) << 32) | ec, kind="stable")
    tile_s = tile[order]
    ec_s = ec[order]
    ev_s = ev[order]
    rloc_s = er[order] & 127

    n_t = np.bincount(tile_s, minlength=cfg.NT)
    aonly = np.bincount(tile[cat == 0], minlength=cfg.NT)
    bonly = np.bincount(tile[cat == 2], minlength=cfg.NT)
    ct = np.maximum((n_t + 127) >> 7, ((aonly + 127) >> 7) + ((bonly + 127) >> 7))
    cB = (bonly + 127) >> 7
    cA = ct - cB
    if not ((cA * 128 >= aonly).all() and (cA * 128 + cB * 128 >= n_t).all()):
        raise RuntimeError("tile chunk packing infeasible for this edge list")
    nB_t = np.maximum(bonly, n_t - cA * 128)
    nA_t = n_t - nB_t

    chstart = np.zeros(cfg.NT + 1, np.int64)
    np.cumsum(ct, out=chstart[1:])
    baseA = np.zeros(cfg.NT + 1, np.int64)
    np.cumsum(cA, out=baseA[1:])
    baseB = np.zeros(cfg.NT + 1, np.int64)
    np.cumsum(cB, out=baseB[1:])
    NCHV = int(chstart[-1])
    NCHV_PAD = _rup(NCHV, PB)
    NIDXA_PAD = _rup(int(baseA[-1]) * 128, cfg.GA_CALL)
    NIDXB_PAD = _rup(max(int(baseB[-1]), 1) * 128, cfg.GB_CALL)

    cum = np.zeros(cfg.NT + 1, np.int64)
    np.cumsum(n_t, out=cum[1:])
    pos = np.arange(E, dtype=np.int64) - cum[tile_s]
    isA = pos < nA_t[tile_s]

    idxA = np.zeros(NIDXA_PAD, np.int16)
    idxB = np.zeros(NIDXB_PAD, np.int16)
    prloc = np.zeros((128, NCHV_PAD), np.float32)
    pval = np.zeros((128, NCHV_PAD), np.float32)

    sA = pos[isA]
    tA = tile_s[isA]
    laneA = (sA & 127).astype(np.int64)
    jA = sA >> 7
    idxA[(baseA[tA] + jA) * 128 + laneA] = ec_s[isA].astype(np.int16)
    gchA = chstart[tA] + jA
    prloc[laneA, gchA] = rloc_s[isA]
    pval[laneA, gchA] = ev_s[isA]

    nb = ~isA
    sB = (pos - nA_t[tile_s])[nb]
    tB = tile_s[nb]
    laneB = (sB & 127).astype(np.int64)
    jB = sB >> 7
    idxB[(baseB[tB] + jB) * 128 + laneB] = (ec_s[nb] - cfg.BBASE).astype(np.int16)
    gchB = chstart[tB] + cA[tB] + jB
    prloc[laneB, gchB] = rloc_s[nb]
    pval[laneB, gchB] = ev_s[nb]

    meta = {
        "cA": tuple(int(v) for v in cA),
        "cB": tuple(int(v) for v in cB),
        "chstart": tuple(int(v) for v in chstart),
        "baseA": tuple(int(v) for v in baseA),
        "baseB": tuple(int(v) for v in baseB),
        "NCHV_PAD": NCHV_PAD,
        "NIDXA_PAD": NIDXA_PAD,
        "NIDXB_PAD": NIDXB_PAD,
    }
    arrays = {
        "idxA": np.ascontiguousarray(idxA.reshape(-1, 16).T),   # [16, NIDXA_PAD/16]
        "idxB": np.ascontiguousarray(idxB.reshape(-1, 16).T),
        "prloc": prloc,
        "pval": pval,
    }
    return arrays, meta


def prep_w(W):
    """W [F*K, F] (rows fin*K + k) -> k-major stack [K*F, F] (rows k*F + fin)."""
    Wk = np.asarray(W).astype(np.float32).reshape(F, K, F).transpose(1, 0, 2)
    return np.ascontiguousarray(Wk.reshape(K * F, F)).astype(ml_dtypes.bfloat16)


# ---------------------------------------------------------------- device IR
def build_nc(cfg, meta, repeat=1, ablate=(), nq=4):
    nc = bacc.Bacc(None, target_bir_lowering=False, debug=False,
                   dynamic_dma_scratch_size=16384, num_swdge_queues=nq)
    dt = mybir.dt
    G = cfg.G
    aluop = mybir.AluOpType
    cA, cB = meta["cA"], meta["cB"]
    chstart, baseA, baseB = meta["chstart"], meta["baseA"], meta["baseB"]
    NCHV_PAD = meta["NCHV_PAD"]
    NIDXA, NIDXB = meta["NIDXA_PAD"], meta["NIDXB_PAD"]

    xb = nc.declare_dram_parameter("xb", [cfg.M, F], dt.bfloat16, isOutput=False)
    idxA_d = nc.declare_dram_parameter("idxA", [16, NIDXA // 16], dt.int16, isOutput=False)
    idxB_d = nc.declare_dram_parameter("idxB", [16, NIDXB // 16], dt.int16, isOutput=False)
    prloc_d = nc.declare_dram_parameter("prloc", [128, NCHV_PAD], dt.float32, isOutput=False)
    pval_d = nc.declare_dram_parameter("pval", [128, NCHV_PAD], dt.float32, isOutput=False)
    wst_d = nc.declare_dram_parameter("wst", [K * F, F], dt.bfloat16, isOutput=False)
    out_d = nc.declare_dram_parameter("out", [cfg.MPAD, F], dt.bfloat16, isOutput=True)

    xs = [nc.dram_tensor(f"xs{k}", [cfg.MPAD, F], dt.float32) for k in range(K - 1)]
    xT_d = nc.dram_tensor("xT", [K * F, cfg.MPAD], dt.bfloat16)
    patd = nc.dram_tensor("patd", [NCHV_PAD * 128, 128], dt.bfloat16)

    CPG_A = cfg.GA_CALL // 128       # chunks per A gather call
    CPG_B = cfg.GB_CALL // 128
    NCALL_A = NIDXA // cfg.GA_CALL
    NCALL_B = NIDXB // cfg.GB_CALL
    NG0 = cfg.MPAD // (128 * G)      # stage0 groups
    WMAX = max(chstart[g * G + G] - chstart[g * G] for g in range(cfg.NGRP))

    with TileContext(nc) as tc:
        with (
            tc.tile_pool(name="io", bufs=1) as io,
            tc.tile_pool(name="patp", bufs=2) as patp,
            tc.tile_pool(name="ga", bufs=2) as gap,
            tc.tile_pool(name="gb", bufs=2) as gbp,
            tc.tile_pool(name="ev", bufs=2) as evp,
            tc.tile_pool(name="prj", bufs=2) as prjp,
            tc.tile_pool(name="ps", bufs=3, space="PSUM") as psp,
            tc.tile_pool(name="psT", bufs=2, space="PSUM") as psTp,
            tc.tile_pool(name="psg", bufs=2, space="PSUM") as psgp,
        ):
            # ---- resident tiles
            idxA_t = io.tile([128, NIDXA // 16], dt.int16)
            idxB_t = io.tile([128, NIDXB // 16], dt.int16)
            prlocT = io.tile([128, NCHV_PAD], dt.float32)
            pvalT = io.tile([128, NCHV_PAD], dt.float32)
            wsb = io.tile([128, K * F // 128, F], dt.bfloat16)
            iota_i = io.tile([128, 128], dt.int16)
            iota_b = io.tile([128, 128], dt.float32)
            pcol_i = io.tile([128, 1], dt.int16)
            pcol_b = io.tile([128, 1], dt.float32)
            ident_t = io.tile([128, 128], dt.bfloat16)

            for i in range(8):
                nc.sync.dma_start(out=idxA_t[16 * i:16 * (i + 1), :], in_=idxA_d[:])
                nc.sync.dma_start(out=idxB_t[16 * i:16 * (i + 1), :], in_=idxB_d[:])
            nc.sync.dma_start(out=prlocT[:], in_=prloc_d[:])
            nc.sync.dma_start(out=pvalT[:], in_=pval_d[:])
            nc.sync.dma_start(out=wsb[:], in_=wst_d[:].rearrange("(j p) f -> p j f", p=128))
            nc.gpsimd.iota(iota_i[:], pattern=[[1, 128]], base=0, channel_multiplier=0)
            nc.vector.tensor_copy(iota_b[:], iota_i[:])
            nc.gpsimd.iota(pcol_i[:], pattern=[[0, 1]], base=0, channel_multiplier=1)
            nc.vector.tensor_copy(pcol_b[:], pcol_i[:])
            nc.vector.tensor_scalar(ident_t[:], iota_b[:], pcol_b[:, 0:1], None,
                                    op0=aluop.is_equal)

            patd_v = patd[:].rearrange("(c p) r -> p c r", p=128)

            gshA = gshB = None
            if "gather" in ablate:
                gshA = io.tile([128, CPG_A, F], dt.bfloat16)
                nc.vector.memset(gshA[:], 0.0)
                gshB = io.tile([128, CPG_B, F], dt.bfloat16)
                nc.vector.memset(gshB[:], 0.0)

            def body():
                # ---- pattern build: pat[lane, r] = (r == rloc[lane]) * val[lane]
                for c0 in range(0, NCHV_PAD, PB):
                    pt = patp.tile([128, max(PB, WMAX), 128], dt.bfloat16, tag="pat")
                    for j in range(PB):
                        nc.vector.tensor_scalar(
                            pt[:, j, :], iota_b[:], prlocT[:, c0 + j:c0 + j + 1],
                            pvalT[:, c0 + j:c0 + j + 1],
                            op0=aluop.is_equal, op1=aluop.mult)
                    nc.sync.dma_start(out=patd_v[:, c0:c0 + PB, :], in_=pt[:, :PB, :])

                # ---- stage0: xb -> xs[0] (f32, zero-padded) + xT stripe 0
                for g in range(NG0):
                    r0 = g * 128 * G
                    nreal = min(max(cfg.M - r0, 0), 128 * G)
                    t0 = evp.tile([128, G, F], dt.bfloat16, tag="t0")
                    if nreal < 128 * G:
                        nc.vector.memset(t0[:], 0.0)
                    ft = nreal // 128
                    if ft:
                        nc.sync.dma_start(
                            out=t0[:, :ft, :],
                            in_=xb[r0:r0 + 128 * ft, :].rearrange("(a p) f -> p a f", p=128))
                    rem = nreal % 128
                    if rem:
                        nc.sync.dma_start(out=t0[:rem, ft, :],
                                          in_=xb[r0 + 128 * ft:r0 + nreal, :])
                    t0f = evp.tile([128, G, F], dt.float32, tag="t0f")
                    nc.vector.tensor_copy(t0f[:], t0[:])
                    nc.sync.dma_start(
                        out=xs[0][r0:r0 + 128 * G, :].rearrange("(a p) f -> p a f", p=128),
                        in_=t0f[:])
                    tp = psTp.tile([64, G, 128], dt.bfloat16, tag="tp")
                    for t in range(G):
                        nc.tensor.transpose(tp[:, t, :], t0[:, t, :], ident_t[:])
                    tps = evp.tile([64, G, 128], dt.bfloat16, tag="tps")
                    nc.vector.tensor_copy(tps[:], tp[:])
                    nc.sync.dma_start(out=xT_d[0:F, r0:r0 + 128 * G], in_=tps[:])

                # ---- Chebyshev steps
                for k in range(1, K):
                    src = xs[k - 1]
                    GAB, GBB = [], []
                    if "gather" in ablate:
                        GAB = [gshA] * NCALL_A
                        GBB = [gshB] * NCALL_B
                    else:
                        for ci in range(NCALL_A):
                            gt = gap.tile([128, CPG_A, F], dt.float32, tag="ga")
                            nc.gpsimd.dma_gather(
                                out_ap=gt[:], in_ap=src[0:cfg.ASPLIT, :],
                                idxs_ap=idxA_t[:, ci * (cfg.GA_CALL // 16):(ci + 1) * (cfg.GA_CALL // 16)],
                                num_idxs=cfg.GA_CALL, num_idxs_reg=cfg.GA_CALL,
                                elem_size=F, single_packet=False, queue_num=ci % nq)
                            gtb = gap.tile([128, CPG_A, F], dt.bfloat16, tag="gab")
                            nc.vector.tensor_copy(gtb[:], gt[:])
                            GAB.append(gtb)
                        for ci in range(NCALL_B):
                            gt = gbp.tile([128, CPG_B, F], dt.float32, tag="gb")
                            nc.gpsimd.dma_gather(
                                out_ap=gt[:], in_ap=src[cfg.BBASE:cfg.MPAD, :],
                                idxs_ap=idxB_t[:, ci * (cfg.GB_CALL // 16):(ci + 1) * (cfg.GB_CALL // 16)],
                                num_idxs=cfg.GB_CALL, num_idxs_reg=cfg.GB_CALL,
                                elem_size=F, single_packet=False, queue_num=(ci + 1) % nq)
                            gtb = gbp.tile([128, CPG_B, F], dt.bfloat16, tag="gbb")
                            nc.vector.tensor_copy(gtb[:], gt[:])
                            GBB.append(gtb)

                    for grp in range(cfg.NGRP):
                        r0 = grp * 128 * G
                        ch0 = chstart[grp * G]
                        w = chstart[grp * G + G] - ch0
                        if w:
                            pt = patp.tile([128, max(PB, WMAX), 128], dt.bfloat16, tag="pat")
                            nc.sync.dma_start(out=pt[:, :w, :],
                                              in_=patd_v[:, ch0:ch0 + w, :])
                        ps = psp.tile([128, G, F], dt.float32, tag="ps")
                        for t in range(G):
                            tid = grp * G + t
                            nch = cA[tid] + cB[tid]
                            if nch == 0:
                                nc.vector.memset(ps[:, t, :], 0.0)
                                continue
                            for j in range(nch):
                                if j < cA[tid]:
                                    ga_i = baseA[tid] + j
                                    mov = GAB[ga_i // CPG_A][:, ga_i % CPG_A, :]
                                else:
                                    gb_i = baseB[tid] + (j - cA[tid])
                                    mov = GBB[gb_i // CPG_B][:, gb_i % CPG_B, :]
                                nc.tensor.matmul(
                                    ps[:, t, :], pt[:, chstart[tid] - ch0 + j, :], mov,
                                    start=(j == 0), stop=(j == nch - 1))
                        xc = evp.tile([128, G, F], dt.float32, tag="xc")
                        nc.sync.dma_start(
                            out=xc[:],
                            in_=src[r0:r0 + 128 * G, :].rearrange("(a p) f -> p a f", p=128))
                        xk_t = evp.tile([128, G, F], dt.float32, tag="xk")
                        if k == 1:
                            nc.vector.tensor_sub(xk_t[:], ps[:], xc[:])
                        else:
                            xp = evp.tile([128, G, F], dt.float32, tag="xp")
                            nc.sync.dma_start(
                                out=xp[:],
                                in_=xs[k - 2][r0:r0 + 128 * G, :].rearrange("(a p) f -> p a f", p=128))
                            tmp = evp.tile([128, G, F], dt.float32, tag="tmp")
                            nc.vector.tensor_sub(tmp[:], ps[:], xc[:])
                            nc.vector.scalar_tensor_tensor(
                                xk_t[:], tmp[:], 2.0, xp[:],
                                op0=aluop.mult, op1=aluop.subtract)
                        if k < K - 1:
                            nc.sync.dma_start(
                                out=xs[k][r0:r0 + 128 * G, :].rearrange("(a p) f -> p a f", p=128),
                                in_=xk_t[:])
                        xkb = evp.tile([128, G, F], dt.bfloat16, tag="xkb")
                        nc.vector.tensor_copy(xkb[:], xk_t[:])
                        tp = psTp.tile([64, G, 128], dt.bfloat16, tag="tp")
                        for t in range(G):
                            nc.tensor.transpose(tp[:, t, :], xkb[:, t, :], ident_t[:])
                        tps = evp.tile([64, G, 128], dt.bfloat16, tag="tps")
                        nc.vector.tensor_copy(tps[:], tp[:])
                        nc.sync.dma_start(out=xT_d[k * F:(k + 1) * F, r0:r0 + 128 * G],
                                          in_=tps[:])

                # ---- dense projection: out = X_cat @ W  (3 stat chunks of 128)
                for grp in range(cfg.NGRP):
                    r0 = grp * 128 * G
                    stx = prjp.tile([128, 3, G * 128], dt.bfloat16, tag="stx")
                    for j in range(3):
                        nc.sync.dma_start(out=stx[:, j, :],
                                          in_=xT_d[128 * j:128 * (j + 1), r0:r0 + 128 * G])
                    pg = psgp.tile([128, G, F], dt.float32, tag="pg")
                    for t in range(G):
                        for j in range(3):
                            nc.tensor.matmul(pg[:, t, :], stx[:, j, t * 128:(t + 1) * 128],
                                             wsb[:, j, :], start=(j == 0), stop=(j == 2))
                    ob = prjp.tile([128, G, F], dt.bfloat16, tag="ob")
                    nc.vector.tensor_copy(ob[:], pg[:])
                    nc.sync.dma_start(
                        out=out_d[r0:r0 + 128 * G, :].rearrange("(a p) f -> p a f", p=128),
                        in_=ob[:])

            for _rep in range(repeat):
                body()

    nc.finalize()
    return nc


_NC_CACHE = {}


def get_nc(cfg, meta, repeat=1, **kw):
    key = (cfg.M, cfg.MPAD, cfg.G, repeat, meta["chstart"], tuple(sorted(kw.items())))
    if key not in _NC_CACHE:
        _NC_CACHE[key] = build_nc(cfg, meta, repeat, **kw)
    return _NC_CACHE[key]


# ---------------------------------------------------------------- entry
def make_in_maps(cfg, x, edge_vals, W, edge_rows, edge_cols):
    arrays, meta = prep_graph(cfg, edge_rows, edge_cols, edge_vals)
    wst = prep_w(W)
    xb16 = np.asarray(x).astype(ml_dtypes.bfloat16)
    in_maps = []
    for c in range(NCORE):
        in_maps.append({
            "xb": xb16[c],
            "idxA": arrays["idxA"],
            "idxB": arrays["idxB"],
            "prloc": arrays["prloc"],
            "pval": arrays["pval"],
            "wst": wst,
        })
    return in_maps, meta


def run(cfg, x, edge_vals, W, edge_rows, edge_cols):
    in_maps, meta = make_in_maps(cfg, x, edge_vals, W, edge_rows, edge_cols)
    nc = get_nc(cfg, meta)
    results = bass2jax.run_bass_via_pjrt(nc, in_maps, n_cores=NCORE)
    Bd = np.asarray(x).shape[0]
    out = np.empty((Bd, cfg.M, F), np.float32)
    for c in range(Bd):
        out[c] = results[c]["out"][:cfg.M].astype(np.float32)
    return out


def kernel(**inputs):
    return run(CFG_FULL, inputs["x"], inputs["edge_vals"], inputs["W"],
               inputs["edge_rows"], inputs["edge_cols"])
